# revision 10
# baseline (speedup 1.0000x reference)
"""Multi-head attention (B=2, S=2048, d_model=768, H=12) on 8 TRN2 NeuronCores.

Sharding: 2-way data parallel over batch x 4-way tensor parallel over heads
(3 heads / 192-wide d_model slice per core). Host compacts masked keys away
(gather of unmasked key/value rows), pads to a 128 multiple, and passes a 0/1
validity vector; softmax needs no mask handling on device (pad keys get V=0
and a 0 in the denominator ones-column).

v2 pipeline (single fused phase, ACT-exp paced):
  - PE warmup matmuls + ACT exp-table preload run during the initial DMAs so
    the HAM clock gate opens (2.4 GHz) before real work.
  - K proj starts as soon as the first XK column slices land; Q proj chunk 0
    follows; remaining Q chunks, all V-proj tiles and the output projection
    are emitted as fillers inside the attention loop (PE slack under the
    ACT-bound exp stream).
  - Scores for heads 0/1 pair into disjoint PE row groups; head 2 pairs two
    key tiles the same way using partition-duplicated K/Q (dup via SBUF DMA).
  - PSUM: ring pool 2x[128,1024] (scores dbl-buffer + Q/K-proj psums + O-proj
    accumulators) = 4 banks, ctx pool 3x[128,512] = 3 banks, junk/V-proj
    1x[128,512] = 1 bank; 8 banks exactly.
  - V blocks per head: h0 [V(0:64)|ones(64)|0], h1/h2 [ones(0)|0|V(64:128)]
    so h1's ctx lands on PSUM partitions 64:128 (no partition shift for the
    normalize multiply into CTX01 rows 64:128); same for h2 into CTX2d.
  - Softmax normalize: DVE reciprocal (from PSUM denom row) -> gpsimd
    partition_broadcast -> DVE multiply (PSUM ctx x bcast -> SBUF bf16).
  - O proj per query tile: CTX01 (128-contraction) + head-2 split across row
    groups (CTX2d lo/hi vs WO2d lo/hi) for partial concurrency.
"""

import math
import os

import numpy as np

B = 2
S = 2048
DM = 768
H = 12
DH = 64
G = 4              # head-group (tensor-parallel) degree
HPG = H // G       # heads per core
DQ = HPG * DH      # 192 d_model slice per core
NCORES = 8
P = 128

WARMUP_MM = 40

_prog_cache = {}


def _chunks(total, step):
    out = []
    o = 0
    while o < total:
        w = min(step, total - o)
        out.append((o, w))
        o += w
    return out


def _groups(n, g):
    out = []
    o = 0
    while o < n:
        out.append(list(range(o, min(o + g, n))))
        o += g
    return out


def _build_nc(KP):
    import concourse.bass as bass
    import concourse.mybir as mybir
    import concourse.tile as tile
    from concourse import bacc

    F32 = mybir.dt.float32
    BF = mybir.dt.bfloat16
    AFT = mybir.ActivationFunctionType

    T = KP // P            # key tiles
    NKT = DM // P          # 6 contraction tiles for projections
    KCH = _chunks(KP, 512)
    NQC = S // 512         # 4 query chunks of 512

    DBG = os.environ.get("KDBG", "0") == "1"

    nc = bacc.Bacc(None, target_bir_lowering=False)
    xqT = nc.declare_dram_parameter("xqT", [DM, S], BF, isOutput=False)
    xkT = nc.declare_dram_parameter("xkT", [DM, KP], BF, isOutput=False)
    xvT = nc.declare_dram_parameter("xvT", [DM, KP], BF, isOutput=False)
    wq = nc.declare_dram_parameter("wq", [DM, DQ], BF, isOutput=False)
    wk = nc.declare_dram_parameter("wk", [DM, DQ], BF, isOutput=False)
    wv = nc.declare_dram_parameter("wv", [DM, DQ], BF, isOutput=False)
    wo = nc.declare_dram_parameter("wo", [DQ, DM], BF, isOutput=False)
    bq = nc.declare_dram_parameter("bq", [DQ, 1], F32, isOutput=False)
    bk = nc.declare_dram_parameter("bk", [DQ, 1], F32, isOutput=False)
    bv = nc.declare_dram_parameter("bv", [1, DQ], F32, isOutput=False)
    vm = nc.declare_dram_parameter("vm", [P, T], F32, isOutput=False)
    out = nc.declare_dram_parameter("out", [S, DM], F32, isOutput=True)
    if DBG:
        d_qt1d = nc.declare_dram_parameter("d_qt1d", [P, S], BF, isOutput=True)
        d_kt1d = nc.declare_dram_parameter("d_kt1d", [P, KP], BF, isOutput=True)
        d_ctx2d = nc.declare_dram_parameter("d_ctx2d", [P, S], BF, isOutput=True)
        d_qt0 = nc.declare_dram_parameter("d_qt0", [P, S], BF, isOutput=True)
        d_kt0 = nc.declare_dram_parameter("d_kt0", [P, KP], BF, isOutput=True)
        d_bc = nc.declare_dram_parameter("d_bc", [P, 3 * 512], F32, isOutput=True)
        d_vp = nc.declare_dram_parameter("d_vp", [P, T * HPG * P], BF, isOutput=True)
        d_ctx01 = nc.declare_dram_parameter("d_ctx01", [P, S], BF, isOutput=True)

    with tile.TileContext(nc) as tc:
        with (
            tc.tile_pool(name="persist", bufs=1) as persist,
            tc.tile_pool(name="es", bufs=4) as espool,
            tc.tile_pool(name="norm", bufs=3) as norm,
            tc.tile_pool(name="osb", bufs=3) as osb,
            tc.tile_pool(name="ring_ps", bufs=2, space="PSUM") as ring_ps,
            tc.tile_pool(name="ctx_ps", bufs=3, space="PSUM") as ctx_ps,
            tc.tile_pool(name="jnk_ps", bufs=1, space="PSUM") as jnk_ps,
        ):
            # ---- weights / constants / activations ----
            WK = persist.tile([P, NKT, DQ], BF, tag="WK")
            WQ = persist.tile([P, NKT, DQ], BF, tag="WQ")
            WV = persist.tile([P, NKT, DQ], BF, tag="WV")
            WO0 = persist.tile([P, DM], BF, tag="WO0")    # wo rows 0:128 (h0,h1)
            WO2d = persist.tile([P, DM], BF, tag="WO2d")  # wo rows 128:192 dup'd
            BQ0 = persist.tile([P, 1], F32, tag="BQ0")
            BQ1 = persist.tile([DH, 1], F32, tag="BQ1")
            BK0 = persist.tile([P, 1], F32, tag="BK0")
            BK1 = persist.tile([DH, 1], F32, tag="BK1")
            BV = persist.tile([P, DQ], F32, tag="BV")
            VM = persist.tile([P, T], F32, tag="VM")
            XK = [persist.tile([P, KP], BF, tag=f"xk{kt}", name=f"xk{kt}")
                  for kt in range(NKT)]
            XQ = [persist.tile([P, S], BF, tag=f"xq{kt}", name=f"xq{kt}")
                  for kt in range(NKT)]
            XV = [persist.tile([P, KP], BF, tag=f"xv{kt}", name=f"xv{kt}")
                  for kt in range(NKT)]

            QT0 = persist.tile([P, S], BF, tag="QT0")     # heads 0,1 q-dims
            QT1d = persist.tile([P, S], BF, tag="QT1d")   # head 2 (rows dup'd)
            KT0 = persist.tile([P, KP], BF, tag="KT0")
            KT1d = persist.tile([P, KP], BF, tag="KT1d")
            VP = persist.tile([P, T, HPG * P], BF, tag="VP")
            CTX01 = persist.tile([P, S], BF, tag="CTX01")
            CTX2d = persist.tile([P, S], BF, tag="CTX2d")
            JNK = persist.tile([P, 512], BF, tag="JNK")

            # ---- DMAs, arrival-ordered ----
            nc.sync.dma_start(out=WK, in_=wk[:, :].rearrange("(kt p) m -> p kt m", p=P))
            nc.sync.dma_start(out=WQ, in_=wq[:, :].rearrange("(kt p) m -> p kt m", p=P))
            nc.sync.dma_start(out=BK0, in_=bk[0:P, :])
            nc.sync.dma_start(out=BK1, in_=bk[P:DQ, :])
            nc.sync.dma_start(out=BQ0, in_=bq[0:P, :])
            nc.sync.dma_start(out=BQ1, in_=bq[P:DQ, :])
            nc.sync.dma_start(out=VM, in_=vm[:, :])
            for kt in range(NKT):
                nc.sync.dma_start(out=XK[kt][:, 0:512], in_=xkT[kt * P:(kt + 1) * P, 0:512])
            for kt in range(NKT):
                nc.sync.dma_start(out=XK[kt][:, 512:KP], in_=xkT[kt * P:(kt + 1) * P, 512:KP])
            for kt in range(NKT):
                nc.sync.dma_start(out=XQ[kt][:, 0:512], in_=xqT[kt * P:(kt + 1) * P, 0:512])
            nc.sync.dma_start(out=WV, in_=wv[:, :].rearrange("(kt p) m -> p kt m", p=P))
            nc.sync.dma_start(out=BV, in_=bv[:, :].to_broadcast([P, DQ]))
            for kt in range(NKT):
                nc.sync.dma_start(out=XV[kt][:, 0:2 * P], in_=xvT[kt * P:(kt + 1) * P, 0:2 * P])
            for kt in range(NKT):
                nc.sync.dma_start(out=XV[kt][:, 2 * P:KP], in_=xvT[kt * P:(kt + 1) * P, 2 * P:KP])
            for kt in range(NKT):
                nc.sync.dma_start(out=XQ[kt][:, 512:S], in_=xqT[kt * P:(kt + 1) * P, 512:S])
            nc.sync.dma_start(out=WO0, in_=wo[0:P, :])
            nc.sync.dma_start(out=WO2d[0:DH, :], in_=wo[P:DQ, :])
            nc.sync.dma_start(out=WO2d[DH:P, :], in_=wo[P:DQ, :])

            # ---- warmup: exp-table preload + PE clock-gate opener ----
            nc.vector.memset(JNK, 0.0)
            jo = norm.tile([1, 16], F32, tag="jo", name="jo")
            nc.scalar.activation(jo, JNK[0:1, 0:16], AFT.Exp, bias=0.0, scale=1.0)
            nc.vector.memset(VP, 0.0)
            for i in range(WARMUP_MM):
                jp = jnk_ps.tile([P, 512], F32, tag="jnk", name=f"warm{i}")
                nc.tensor.matmul(jp[:, 0:P], lhsT=JNK[:, 0:P], rhs=JNK[:, 0:P],
                                 start=True, stop=True)

            # ---- K projection (all chunks) ----
            for (ck, cwk) in KCH:
                sl = ring_ps.tile([P, 1024], F32, tag="ring", name=f"kps{ck}")
                for kt in range(NKT):
                    nc.tensor.matmul(sl[:, 0:cwk], lhsT=WK[:, kt, 0:P],
                                     rhs=XK[kt][:, ck:ck + cwk],
                                     start=(kt == 0), stop=(kt == NKT - 1))
                for kt in range(NKT):
                    nc.tensor.matmul(sl[0:DH, 512:512 + cwk], lhsT=WK[:, kt, P:DQ],
                                     rhs=XK[kt][:, ck:ck + cwk],
                                     start=(kt == 0), stop=(kt == NKT - 1))
                nc.scalar.activation(KT0[:, ck:ck + cwk], sl[:, 0:cwk],
                                     AFT.Identity, bias=BK0)
                nc.scalar.activation(KT1d[0:DH, ck:ck + cwk], sl[0:DH, 512:512 + cwk],
                                     AFT.Identity, bias=BK1)
                nc.sync.dma_start(out=KT1d[DH:P, ck:ck + cwk], in_=KT1d[0:DH, ck:ck + cwk])

            # ---- Q projection (chunk 0 on ACT; later chunks on DVE as fillers)
            def q_proj(ci, on_act):
                cq = ci * 512
                sl = ring_ps.tile([P, 1024], F32, tag="ring", name=f"qps{ci}")
                for kt in range(NKT):
                    nc.tensor.matmul(sl[:, 0:512], lhsT=WQ[:, kt, 0:P],
                                     rhs=XQ[kt][:, cq:cq + 512],
                                     start=(kt == 0), stop=(kt == NKT - 1))
                for kt in range(NKT):
                    nc.tensor.matmul(sl[0:DH, 512:1024], lhsT=WQ[:, kt, P:DQ],
                                     rhs=XQ[kt][:, cq:cq + 512],
                                     start=(kt == 0), stop=(kt == NKT - 1))
                if on_act:
                    nc.scalar.activation(QT0[:, cq:cq + 512], sl[:, 0:512],
                                         AFT.Identity, bias=BQ0)
                    nc.scalar.activation(QT1d[0:DH, cq:cq + 512], sl[0:DH, 512:1024],
                                         AFT.Identity, bias=BQ1)
                else:
                    nc.vector.tensor_scalar_add(QT0[:, cq:cq + 512], sl[:, 0:512], BQ0)
                    nc.vector.tensor_scalar_add(QT1d[0:DH, cq:cq + 512],
                                                sl[0:DH, 512:1024], BQ1)
                nc.sync.dma_start(out=QT1d[DH:P, cq:cq + 512], in_=QT1d[0:DH, cq:cq + 512])

            q_proj(0, True)

            # ---- V projection tile -> VP block build ----
            # block layout per t (384 cols): h0 [V(0:64)|ones(64)|0],
            # h1 [ones(128)|0|V(192:256)], h2 [ones(256)|0|V(320:384)]
            def v_proj(t):
                ps = jnk_ps.tile([P, 512], F32, tag="jnk", name=f"vps{t}")
                for kt in range(NKT):
                    nc.tensor.matmul(ps[:, 0:DQ], lhsT=XV[kt][:, t * P:(t + 1) * P],
                                     rhs=WV[:, kt, :],
                                     start=(kt == 0), stop=(kt == NKT - 1))
                blk = VP[:, t, :]
                hview = VP[:, t, :].rearrange("p (h c) -> p h c", c=P)  # [P,3,128]
                v12 = hview[:, 1:3, DH:P]   # abs cols {192:256, 320:384}
                nc.vector.tensor_add(blk[:, 0:DH], ps[:, 0:DH],
                                     BV[:, 0:DH])
                nc.vector.tensor_add(
                    v12,
                    ps[:, DH:DQ].rearrange("p (h d) -> p h d", d=DH),
                    BV[:, DH:DQ].rearrange("p (h d) -> p h d", d=DH))
                nc.vector.tensor_scalar_mul(blk[:, 0:DH], blk[:, 0:DH], VM[:, t:t + 1])
                nc.vector.tensor_scalar_mul(v12, v12, VM[:, t:t + 1])
                nc.vector.tensor_copy(blk[:, DH:DH + 1], VM[:, t:t + 1])
                nc.vector.tensor_copy(
                    hview[:, 1:3, 0:1],
                    VM[:, t:t + 1].to_broadcast([P, 2, 1]))

            v_proj(0)
            v_proj(1)

            # ---- O projection for one query tile (ring psum) ----
            def o_proj(m):
                po = ring_ps.tile([P, 1024], F32, tag="ring", name=f"po{m}")
                ctxm = CTX01[:, m * P:(m + 1) * P]
                nc.tensor.matmul(po[:, 0:512], lhsT=ctxm, rhs=WO0[:, 0:512],
                                 start=True, stop=False)
                nc.tensor.matmul(po[:, 512:DM], lhsT=ctxm, rhs=WO0[:, 512:DM],
                                 start=True, stop=False)
                nc.tensor.matmul(po[:, 0:512], lhsT=CTX2d[0:DH, m * P:(m + 1) * P],
                                 rhs=WO2d[0:DH, 0:512], start=False, stop=True)
                nc.tensor.matmul(po[:, 512:DM], lhsT=CTX2d[DH:P, m * P:(m + 1) * P],
                                 rhs=WO2d[DH:P, 512:DM], start=False, stop=True)
                po_sb = osb.tile([P, DM], F32, tag="posb", name=f"posb{m}")
                nc.vector.tensor_copy(po_sb, po[:, 0:DM])
                nc.sync.dma_start(out=out[m * P:(m + 1) * P, :], in_=po_sb)

            # ---- normalize: ctx/denom -> CTX01 / CTX2d (SBUF bf16) ----
            def normalize(ci, ctx0, ctx1, ctx2):
                cq = ci * 512
                dn0 = norm.tile([1, 512], F32, tag="dn", name=f"dn0_{ci}")
                nc.vector.tensor_copy(dn0, ctx0[DH:DH + 1, :])
                rc0 = norm.tile([1, 512], F32, tag="rc", name=f"rc0_{ci}")
                nc.vector.reciprocal_approx_fast(rc0, dn0)
                bc0 = norm.tile([P, 512], F32, tag="bc", name=f"bc0_{ci}")
                nc.gpsimd.partition_broadcast(bc0[0:DH, :], rc0)
                nc.vector.tensor_mul(CTX01[0:DH, cq:cq + 512], ctx0[0:DH, :], bc0[0:DH, :])
                rc1 = norm.tile([1, 512], F32, tag="rc", name=f"rc1_{ci}")
                nc.vector.reciprocal_approx_fast(rc1, ctx1[0:1, :])
                bc1 = norm.tile([P, 512], F32, tag="bc", name=f"bc1_{ci}")
                nc.gpsimd.partition_broadcast(bc1[0:P, :], rc1)
                nc.vector.tensor_mul(CTX01[DH:P, cq:cq + 512], ctx1[DH:P, :], bc1[DH:P, :])
                rc2 = norm.tile([1, 512], F32, tag="rc", name=f"rc2_{ci}")
                nc.vector.reciprocal_approx_fast(rc2, ctx2[0:1, :])
                bc2 = norm.tile([P, 512], F32, tag="bc", name=f"bc2_{ci}")
                nc.gpsimd.partition_broadcast(bc2[0:P, :], rc2)
                nc.vector.tensor_mul(CTX2d[DH:P, cq:cq + 512], ctx2[DH:P, :], bc2[DH:P, :])
                nc.sync.dma_start(out=CTX2d[0:DH, cq:cq + 512], in_=CTX2d[DH:P, cq:cq + 512])
                if DBG and ci == 0:
                    nc.sync.dma_start(out=d_bc[:, 0:512], in_=bc0)
                    nc.sync.dma_start(out=d_bc[:, 512:1024], in_=bc1)
                    nc.sync.dma_start(out=d_bc[:, 1024:1536], in_=bc2)

            # ---- attention main loop, ACT-exp paced ----
            Bg = _groups(T, 2)

            def emit_chunk(ci, fillers):
                cq = ci * 512
                ctx0 = ctx_ps.tile([P, 512], F32, tag="ctx", name=f"c0_{ci}")
                ctx1 = ctx_ps.tile([P, 512], F32, tag="ctx", name=f"c1_{ci}")
                ctx2 = ctx_ps.tile([P, 512], F32, tag="ctx", name=f"c2_{ci}")
                fill = list(fillers)

                acts = []
                bi = 0
                for t in range(T):
                    acts.append(("A", t))
                    if t % 2 == 1 and bi < len(Bg):
                        acts.append(("B", bi))
                        bi += 1
                while bi < len(Bg):
                    acts.append(("B", bi))
                    bi += 1

                def emit_scores(ev):
                    kind, i = ev
                    spt = ring_ps.tile([P, 1024], F32, tag="ring",
                                       name=f"sp{kind}{ci}_{i}")
                    if kind == "A":
                        t = i
                        nc.tensor.matmul(spt[:, 0:512],
                                         lhsT=KT0[0:DH, t * P:(t + 1) * P],
                                         rhs=QT0[0:DH, cq:cq + 512],
                                         start=True, stop=True)
                        nc.tensor.matmul(spt[:, 512:1024],
                                         lhsT=KT0[DH:P, t * P:(t + 1) * P],
                                         rhs=QT0[DH:P, cq:cq + 512],
                                         start=True, stop=True)
                    else:
                        tg = Bg[i]
                        nc.tensor.matmul(spt[:, 0:512],
                                         lhsT=KT1d[0:DH, tg[0] * P:(tg[0] + 1) * P],
                                         rhs=QT1d[0:DH, cq:cq + 512],
                                         start=True, stop=True)
                        if len(tg) == 2:
                            nc.tensor.matmul(spt[:, 512:1024],
                                             lhsT=KT1d[DH:P, tg[1] * P:(tg[1] + 1) * P],
                                             rhs=QT1d[DH:P, cq:cq + 512],
                                             start=True, stop=True)
                    return spt

                def emit_exp(ev, spt):
                    kind, i = ev
                    w = 1024
                    if kind == "B" and len(Bg[i]) == 1:
                        w = 512
                    es = espool.tile([P, 1024], BF, tag="es", name=f"es{kind}{ci}_{i}")
                    nc.scalar.activation(es[:, 0:w], spt[:, 0:w], AFT.Exp,
                                         bias=0.0, scale=1.0 / math.sqrt(DH))
                    return es

                def emit_pv(ev, es):
                    kind, i = ev
                    if kind == "A":
                        t = i
                        nc.tensor.matmul(ctx0[:, :], lhsT=VP[:, t, 0:P],
                                         rhs=es[:, 0:512],
                                         start=(t == 0), stop=(t == T - 1))
                        nc.tensor.matmul(ctx1[:, :], lhsT=VP[:, t, P:2 * P],
                                         rhs=es[:, 512:1024],
                                         start=(t == 0), stop=(t == T - 1))
                    else:
                        for j, t in enumerate(Bg[i]):
                            nc.tensor.matmul(ctx2[:, :], lhsT=VP[:, t, 2 * P:3 * P],
                                             rhs=es[:, j * 512:(j + 1) * 512],
                                             start=(t == 0), stop=(t == T - 1))

                sps = {0: emit_scores(acts[0])}
                for i, ev in enumerate(acts):
                    es = emit_exp(ev, sps.pop(i))
                    if i + 1 < len(acts):
                        sps[i + 1] = emit_scores(acts[i + 1])
                    emit_pv(ev, es)
                    if fill:
                        fill.pop(0)()
                while fill:
                    fill.pop(0)()
                normalize(ci, ctx0, ctx1, ctx2)

            fillers = {
                0: [(lambda t=t: v_proj(t)) for t in range(2, T)] + [lambda: q_proj(1, False)],
                1: [(lambda m=m: o_proj(m)) for m in range(0, 4)] + [lambda: q_proj(2, False)],
                2: [(lambda m=m: o_proj(m)) for m in range(4, 8)] + [lambda: q_proj(3, False)],
                3: [(lambda m=m: o_proj(m)) for m in range(8, 12)],
            }
            for ci in range(NQC):
                emit_chunk(ci, fillers[ci])
            for m in range(12, 16):
                o_proj(m)
            if DBG:
                nc.sync.dma_start(out=d_qt1d[:, :], in_=QT1d)
                nc.sync.dma_start(out=d_kt1d[:, :], in_=KT1d)
                nc.sync.dma_start(out=d_ctx2d[:, :], in_=CTX2d)
                nc.sync.dma_start(out=d_qt0[:, :], in_=QT0)
                nc.sync.dma_start(out=d_kt0[:, :], in_=KT0)
                nc.sync.dma_start(out=d_vp[:, :], in_=VP.rearrange("p t c -> p (t c)"))
                nc.sync.dma_start(out=d_ctx01[:, :], in_=CTX01)

    nc.compile()
    return nc


def _get_prog(KP):
    if KP not in _prog_cache:
        _prog_cache[KP] = _build_nc(KP)
    return _prog_cache[KP]


def _run(inputs, trace=False):
    import ml_dtypes
    from concourse.bass_utils import run_bass_kernel_spmd

    BF = ml_dtypes.bfloat16

    query = np.asarray(inputs["query"], dtype=np.float32)
    key = np.asarray(inputs["key"], dtype=np.float32)
    value = np.asarray(inputs["value"], dtype=np.float32)
    mask = np.asarray(inputs["mask"])
    Wq = np.asarray(inputs["Wq"], dtype=np.float32)
    bq = np.asarray(inputs["bq"], dtype=np.float32)
    Wk = np.asarray(inputs["Wk"], dtype=np.float32)
    bk = np.asarray(inputs["bk"], dtype=np.float32)
    Wv = np.asarray(inputs["Wv"], dtype=np.float32)
    bv = np.asarray(inputs["bv"], dtype=np.float32)
    Wo = np.asarray(inputs["Wo"], dtype=np.float32)
    bo = np.asarray(inputs["bo"], dtype=np.float32)

    idx = [np.nonzero(mask[b, 0, 0] != 0)[0] for b in range(B)]
    keff = [len(i) for i in idx]
    KP = max(P, ((max(keff) + P - 1) // P) * P)
    T = KP // P

    nc = _get_prog(KP)

    per_batch = {}
    for b in range(B):
        xqT = np.ascontiguousarray(query[b].T).astype(BF)
        xkT = np.zeros((DM, KP), dtype=BF)
        xkT[:, :keff[b]] = key[b][idx[b]].T.astype(BF)
        xvT = np.zeros((DM, KP), dtype=BF)
        xvT[:, :keff[b]] = value[b][idx[b]].T.astype(BF)
        vmf = np.zeros((KP,), dtype=np.float32)
        vmf[:keff[b]] = 1.0
        vm2 = np.ascontiguousarray(vmf.reshape(T, P).T)  # [128, T]
        per_batch[b] = (xqT, xkT, xvT, vm2)

    in_maps = []
    for core in range(NCORES):
        b, g = core // G, core % G
        xqT, xkT, xvT, vm2 = per_batch[b]
        sl = slice(g * DQ, (g + 1) * DQ)
        in_maps.append({
            "xqT": xqT,
            "xkT": xkT,
            "xvT": xvT,
            "wq": np.ascontiguousarray(Wq[:, sl]).astype(BF),
            "wk": np.ascontiguousarray(Wk[:, sl]).astype(BF),
            "wv": np.ascontiguousarray(Wv[:, sl]).astype(BF),
            "wo": np.ascontiguousarray(Wo[sl, :]).astype(BF),
            "bq": np.ascontiguousarray(bq[sl].reshape(DQ, 1)),
            "bk": np.ascontiguousarray(bk[sl].reshape(DQ, 1)),
            "bv": np.ascontiguousarray(bv[sl].reshape(1, DQ)),
            "vm": vm2,
        })

    res = run_bass_kernel_spmd(nc, in_maps, list(range(NCORES)), trace=trace)

    outp = np.zeros((B, S, DM), dtype=np.float32)
    for core in range(NCORES):
        outp[core // G] += res.results[core]["out"]
    outp += bo.reshape(1, 1, DM)
    return outp, res


def kernel(**inputs) -> np.ndarray:
    out, _ = _run(inputs, trace=False)
    return out


if __name__ == "__main__":
    nc = _build_nc(1152)
    print("build OK")


# revision 13
# speedup vs baseline: 1.2313x; 1.2313x over previous
"""Multi-head attention (B=2, S=2048, d_model=768, H=12) on 8 TRN2 NeuronCores.

Sharding: 2-way data parallel over batch x 4-way tensor parallel over heads
(3 heads / 192-wide d_model slice per core). Host compacts masked keys away
(gather of unmasked key/value rows), pads to a 128 multiple, and passes a 0/1
validity vector; softmax needs no mask handling on device (pad keys get V=0
and a 0 in the denominator ones-column).

v2 pipeline (single fused phase, ACT-exp paced):
  - PE warmup matmuls + ACT exp-table preload run during the initial DMAs so
    the HAM clock gate opens (2.4 GHz) before real work.
  - K proj starts as soon as the first XK column slices land; Q proj chunk 0
    follows; remaining Q chunks, all V-proj tiles and the output projection
    are emitted as fillers inside the attention loop (PE slack under the
    ACT-bound exp stream).
  - Scores for heads 0/1 pair into disjoint PE row groups; head 2 pairs two
    key tiles the same way using partition-duplicated K/Q (dup via SBUF DMA).
  - PSUM: ring pool 2x[128,1024] (scores dbl-buffer + Q/K-proj psums + O-proj
    accumulators) = 4 banks, ctx pool 3x[128,512] = 3 banks, junk/V-proj
    1x[128,512] = 1 bank; 8 banks exactly.
  - V blocks per head: h0 [V(0:64)|ones(64)|0], h1/h2 [ones(0)|0|V(64:128)]
    so h1's ctx lands on PSUM partitions 64:128 (no partition shift for the
    normalize multiply into CTX01 rows 64:128); same for h2 into CTX2d.
  - Softmax normalize: DVE reciprocal (from PSUM denom row) -> gpsimd
    partition_broadcast -> DVE multiply (PSUM ctx x bcast -> SBUF bf16).
  - O proj per query tile: CTX01 (128-contraction) + head-2 split across row
    groups (CTX2d lo/hi vs WO2d lo/hi) for partial concurrency.
"""

import math
import os

import numpy as np

B = 2
S = 2048
DM = 768
H = 12
DH = 64
G = 4              # head-group (tensor-parallel) degree
HPG = H // G       # heads per core
DQ = HPG * DH      # 192 d_model slice per core
NCORES = 8
P = 128

WARMUP_MM = 40

_prog_cache = {}


def _chunks(total, step):
    out = []
    o = 0
    while o < total:
        w = min(step, total - o)
        out.append((o, w))
        o += w
    return out


def _groups(n, g):
    out = []
    o = 0
    while o < n:
        out.append(list(range(o, min(o + g, n))))
        o += g
    return out


def _build_nc(KP):
    import concourse.bass as bass
    import concourse.mybir as mybir
    import concourse.tile as tile
    from concourse import bacc

    F32 = mybir.dt.float32
    BF = mybir.dt.bfloat16
    AFT = mybir.ActivationFunctionType

    T = KP // P            # key tiles
    NKT = DM // P          # 6 contraction tiles for projections
    KCH = _chunks(KP, 512)
    NQC = S // 512         # 4 query chunks of 512

    DBG = os.environ.get("KDBG", "0") == "1"

    nc = bacc.Bacc(None, target_bir_lowering=False)
    xqT = nc.declare_dram_parameter("xqT", [DM, S], BF, isOutput=False)
    xkT = nc.declare_dram_parameter("xkT", [DM, KP], BF, isOutput=False)
    xvT = nc.declare_dram_parameter("xvT", [DM, KP], BF, isOutput=False)
    # host pre-transposes projection weights to [P, NKT*DQ] (contiguous DMA)
    wq = nc.declare_dram_parameter("wq", [P, NKT * DQ], BF, isOutput=False)
    wk = nc.declare_dram_parameter("wk", [P, NKT * DQ], BF, isOutput=False)
    wv = nc.declare_dram_parameter("wv", [P, NKT * DQ], BF, isOutput=False)
    wo = nc.declare_dram_parameter("wo", [DQ, DM], BF, isOutput=False)
    bq = nc.declare_dram_parameter("bq", [DQ, 1], F32, isOutput=False)
    bk = nc.declare_dram_parameter("bk", [DQ, 1], F32, isOutput=False)
    bv = nc.declare_dram_parameter("bv", [1, DQ], F32, isOutput=False)
    vm = nc.declare_dram_parameter("vm", [P, T], F32, isOutput=False)
    out = nc.declare_dram_parameter("out", [S, DM], F32, isOutput=True)
    if DBG:
        d_qt1d = nc.declare_dram_parameter("d_qt1d", [P, S], BF, isOutput=True)
        d_kt1d = nc.declare_dram_parameter("d_kt1d", [P, KP], BF, isOutput=True)
        d_ctx2d = nc.declare_dram_parameter("d_ctx2d", [P, S], BF, isOutput=True)
        d_qt0 = nc.declare_dram_parameter("d_qt0", [P, S], BF, isOutput=True)
        d_kt0 = nc.declare_dram_parameter("d_kt0", [P, KP], BF, isOutput=True)
        d_bc = nc.declare_dram_parameter("d_bc", [P, 3 * 512], F32, isOutput=True)
        d_vp = nc.declare_dram_parameter("d_vp", [P, T * HPG * P], BF, isOutput=True)
        d_ctx01 = nc.declare_dram_parameter("d_ctx01", [P, S], BF, isOutput=True)

    with tile.TileContext(nc) as tc:
        with (
            tc.tile_pool(name="persist", bufs=1) as persist,
            tc.tile_pool(name="es", bufs=4) as espool,
            tc.tile_pool(name="norm", bufs=3) as norm,
            tc.tile_pool(name="osb", bufs=3) as osb,
            tc.tile_pool(name="ring_ps", bufs=2, space="PSUM") as ring_ps,
            tc.tile_pool(name="ctx_ps", bufs=3, space="PSUM") as ctx_ps,
            tc.tile_pool(name="jnk_ps", bufs=1, space="PSUM") as jnk_ps,
        ):
            # ---- weights / constants / activations ----
            WK = persist.tile([P, NKT, DQ], BF, tag="WK")
            WQ = persist.tile([P, NKT, DQ], BF, tag="WQ")
            WV = persist.tile([P, NKT, DQ], BF, tag="WV")
            WO0 = persist.tile([P, DM], BF, tag="WO0")    # wo rows 0:128 (h0,h1)
            WO2d = persist.tile([P, DM], BF, tag="WO2d")  # wo rows 128:192 dup'd
            BQ0 = persist.tile([P, 1], F32, tag="BQ0")
            BQ1 = persist.tile([DH, 1], F32, tag="BQ1")
            BK0 = persist.tile([P, 1], F32, tag="BK0")
            BK1 = persist.tile([DH, 1], F32, tag="BK1")
            BV = persist.tile([P, DQ], F32, tag="BV")
            VM = persist.tile([P, T], F32, tag="VM")
            XK = [persist.tile([P, KP], BF, tag=f"xk{kt}", name=f"xk{kt}")
                  for kt in range(NKT)]
            XQ = [persist.tile([P, S], BF, tag=f"xq{kt}", name=f"xq{kt}")
                  for kt in range(NKT)]
            XV = [persist.tile([P, KP], BF, tag=f"xv{kt}", name=f"xv{kt}")
                  for kt in range(NKT)]

            QT0 = persist.tile([P, S], BF, tag="QT0")     # heads 0,1 q-dims
            QT1d = persist.tile([P, S], BF, tag="QT1d")   # head 2 (rows dup'd)
            KT0 = persist.tile([P, KP], BF, tag="KT0")
            KT1d = persist.tile([P, KP], BF, tag="KT1d")
            VP = persist.tile([P, T, HPG * P], BF, tag="VP")
            CTX01 = persist.tile([P, S], BF, tag="CTX01")
            CTX2d = persist.tile([P, S], BF, tag="CTX2d")
            JNK = persist.tile([P, 512], BF, tag="JNK")

            # ---- DMAs, arrival-ordered (weights pre-transposed on host) ----
            nc.sync.dma_start(out=WK, in_=wk[:, :].rearrange("p (kt m) -> p kt m", m=DQ))
            nc.sync.dma_start(out=WQ, in_=wq[:, :].rearrange("p (kt m) -> p kt m", m=DQ))
            nc.sync.dma_start(out=BK0, in_=bk[0:P, :])
            nc.sync.dma_start(out=BK1, in_=bk[P:DQ, :])
            nc.sync.dma_start(out=BQ0, in_=bq[0:P, :])
            nc.sync.dma_start(out=BQ1, in_=bq[P:DQ, :])
            nc.sync.dma_start(out=VM, in_=vm[:, :])
            for kt in range(NKT):
                nc.sync.dma_start(out=XK[kt], in_=xkT[kt * P:(kt + 1) * P, :])
            for kt in range(NKT):
                nc.sync.dma_start(out=XQ[kt], in_=xqT[kt * P:(kt + 1) * P, :])
            nc.sync.dma_start(out=WV, in_=wv[:, :].rearrange("p (kt m) -> p kt m", m=DQ))
            nc.sync.dma_start(out=BV, in_=bv[:, :].to_broadcast([P, DQ]))
            for kt in range(NKT):
                nc.sync.dma_start(out=XV[kt], in_=xvT[kt * P:(kt + 1) * P, :])
            nc.sync.dma_start(out=WO0, in_=wo[0:P, :])
            nc.sync.dma_start(out=WO2d[0:DH, :], in_=wo[P:DQ, :])
            nc.sync.dma_start(out=WO2d[DH:P, :], in_=wo[P:DQ, :])

            # ---- warmup: exp-table preload + PE clock-gate opener ----
            nc.vector.memset(JNK, 0.0)
            jo = norm.tile([1, 16], F32, tag="jo", name="jo")
            nc.scalar.activation(jo, JNK[0:1, 0:16], AFT.Exp, bias=0.0, scale=1.0)
            nc.vector.memset(VP, 0.0)
            for i in range(WARMUP_MM):
                jp = jnk_ps.tile([P, 512], F32, tag="jnk", name=f"warm{i}")
                nc.tensor.matmul(jp[:, 0:P], lhsT=JNK[:, 0:P], rhs=JNK[:, 0:P],
                                 start=True, stop=True)

            # ---- K projection (all chunks) ----
            for (ck, cwk) in KCH:
                sl = ring_ps.tile([P, 1024], F32, tag="ring", name=f"kps{ck}")
                for kt in range(NKT):
                    nc.tensor.matmul(sl[:, 0:cwk], lhsT=WK[:, kt, 0:P],
                                     rhs=XK[kt][:, ck:ck + cwk],
                                     start=(kt == 0), stop=(kt == NKT - 1))
                for kt in range(NKT):
                    nc.tensor.matmul(sl[0:DH, 512:512 + cwk], lhsT=WK[:, kt, P:DQ],
                                     rhs=XK[kt][:, ck:ck + cwk],
                                     start=(kt == 0), stop=(kt == NKT - 1))
                nc.scalar.activation(KT0[:, ck:ck + cwk], sl[:, 0:cwk],
                                     AFT.Identity, bias=BK0)
                nc.scalar.activation(KT1d[0:DH, ck:ck + cwk], sl[0:DH, 512:512 + cwk],
                                     AFT.Identity, bias=BK1)
                nc.sync.dma_start(out=KT1d[DH:P, ck:ck + cwk], in_=KT1d[0:DH, ck:ck + cwk])

            # ---- Q projection (chunk 0 on ACT; later chunks on DVE as fillers)
            def q_proj(ci, on_act):
                cq = ci * 512
                sl = ring_ps.tile([P, 1024], F32, tag="ring", name=f"qps{ci}")
                for kt in range(NKT):
                    nc.tensor.matmul(sl[:, 0:512], lhsT=WQ[:, kt, 0:P],
                                     rhs=XQ[kt][:, cq:cq + 512],
                                     start=(kt == 0), stop=(kt == NKT - 1))
                for kt in range(NKT):
                    nc.tensor.matmul(sl[0:DH, 512:1024], lhsT=WQ[:, kt, P:DQ],
                                     rhs=XQ[kt][:, cq:cq + 512],
                                     start=(kt == 0), stop=(kt == NKT - 1))
                if on_act:
                    nc.scalar.activation(QT0[:, cq:cq + 512], sl[:, 0:512],
                                         AFT.Identity, bias=BQ0)
                    nc.scalar.activation(QT1d[0:DH, cq:cq + 512], sl[0:DH, 512:1024],
                                         AFT.Identity, bias=BQ1)
                else:
                    nc.vector.tensor_scalar_add(QT0[:, cq:cq + 512], sl[:, 0:512], BQ0)
                    nc.vector.tensor_scalar_add(QT1d[0:DH, cq:cq + 512],
                                                sl[0:DH, 512:1024], BQ1)
                nc.sync.dma_start(out=QT1d[DH:P, cq:cq + 512], in_=QT1d[0:DH, cq:cq + 512])

            q_proj(0, True)

            # ---- V projection tile -> VP block build ----
            # block layout per t (384 cols): h0 [V(0:64)|ones(64)|0],
            # h1 [ones(128)|0|V(192:256)], h2 [ones(256)|0|V(320:384)]
            def v_proj(t):
                ps = jnk_ps.tile([P, 512], F32, tag="jnk", name=f"vps{t}")
                for kt in range(NKT):
                    nc.tensor.matmul(ps[:, 0:DQ], lhsT=XV[kt][:, t * P:(t + 1) * P],
                                     rhs=WV[:, kt, :],
                                     start=(kt == 0), stop=(kt == NKT - 1))
                blk = VP[:, t, :]
                hview = VP[:, t, :].rearrange("p (h c) -> p h c", c=P)  # [P,3,128]
                v12 = hview[:, 1:3, DH:P]   # abs cols {192:256, 320:384}
                nc.vector.tensor_add(blk[:, 0:DH], ps[:, 0:DH],
                                     BV[:, 0:DH])
                nc.vector.tensor_add(
                    v12,
                    ps[:, DH:DQ].rearrange("p (h d) -> p h d", d=DH),
                    BV[:, DH:DQ].rearrange("p (h d) -> p h d", d=DH))
                nc.vector.tensor_scalar_mul(blk[:, 0:DH], blk[:, 0:DH], VM[:, t:t + 1])
                nc.vector.tensor_scalar_mul(v12, v12, VM[:, t:t + 1])
                nc.vector.tensor_copy(blk[:, DH:DH + 1], VM[:, t:t + 1])
                nc.vector.tensor_copy(
                    hview[:, 1:3, 0:1],
                    VM[:, t:t + 1].to_broadcast([P, 2, 1]))

            v_proj(0)
            v_proj(1)

            # ---- O projection for one query tile (ring psum) ----
            def o_proj(m):
                po = ring_ps.tile([P, 1024], F32, tag="ring", name=f"po{m}")
                ctxm = CTX01[:, m * P:(m + 1) * P]
                nc.tensor.matmul(po[:, 0:512], lhsT=ctxm, rhs=WO0[:, 0:512],
                                 start=True, stop=False)
                nc.tensor.matmul(po[:, 512:DM], lhsT=ctxm, rhs=WO0[:, 512:DM],
                                 start=True, stop=False)
                nc.tensor.matmul(po[:, 0:512], lhsT=CTX2d[0:DH, m * P:(m + 1) * P],
                                 rhs=WO2d[0:DH, 0:512], start=False, stop=True)
                nc.tensor.matmul(po[:, 512:DM], lhsT=CTX2d[DH:P, m * P:(m + 1) * P],
                                 rhs=WO2d[DH:P, 512:DM], start=False, stop=True)
                po_sb = osb.tile([P, DM], F32, tag="posb", name=f"posb{m}")
                nc.vector.tensor_copy(po_sb, po[:, 0:DM])
                nc.sync.dma_start(out=out[m * P:(m + 1) * P, :], in_=po_sb)

            # ---- normalize: ctx/denom -> CTX01 / CTX2d (SBUF bf16) ----
            def normalize(ci, ctx0, ctx1, ctx2):
                cq = ci * 512
                dn0 = norm.tile([1, 512], F32, tag="dn", name=f"dn0_{ci}")
                nc.vector.tensor_copy(dn0, ctx0[DH:DH + 1, :])
                rc0 = norm.tile([1, 512], F32, tag="rc", name=f"rc0_{ci}")
                nc.vector.reciprocal_approx_fast(rc0, dn0)
                bc0 = norm.tile([P, 512], F32, tag="bc", name=f"bc0_{ci}")
                nc.gpsimd.partition_broadcast(bc0[0:DH, :], rc0)
                nc.vector.tensor_mul(CTX01[0:DH, cq:cq + 512], ctx0[0:DH, :], bc0[0:DH, :])
                rc1 = norm.tile([1, 512], F32, tag="rc", name=f"rc1_{ci}")
                nc.vector.reciprocal_approx_fast(rc1, ctx1[0:1, :])
                bc1 = norm.tile([P, 512], F32, tag="bc", name=f"bc1_{ci}")
                nc.gpsimd.partition_broadcast(bc1[0:P, :], rc1)
                nc.vector.tensor_mul(CTX01[DH:P, cq:cq + 512], ctx1[DH:P, :], bc1[DH:P, :])
                rc2 = norm.tile([1, 512], F32, tag="rc", name=f"rc2_{ci}")
                nc.vector.reciprocal_approx_fast(rc2, ctx2[0:1, :])
                bc2 = norm.tile([P, 512], F32, tag="bc", name=f"bc2_{ci}")
                nc.gpsimd.partition_broadcast(bc2[0:P, :], rc2)
                nc.vector.tensor_mul(CTX2d[DH:P, cq:cq + 512], ctx2[DH:P, :], bc2[DH:P, :])
                nc.sync.dma_start(out=CTX2d[0:DH, cq:cq + 512], in_=CTX2d[DH:P, cq:cq + 512])
                if DBG and ci == 0:
                    nc.sync.dma_start(out=d_bc[:, 0:512], in_=bc0)
                    nc.sync.dma_start(out=d_bc[:, 512:1024], in_=bc1)
                    nc.sync.dma_start(out=d_bc[:, 1024:1536], in_=bc2)

            # ---- attention main loop, ACT-exp paced ----
            Bg = _groups(T, 2)

            def emit_chunk(ci, fillers):
                cq = ci * 512
                ctx0 = ctx_ps.tile([P, 512], F32, tag="ctx", name=f"c0_{ci}")
                ctx1 = ctx_ps.tile([P, 512], F32, tag="ctx", name=f"c1_{ci}")
                ctx2 = ctx_ps.tile([P, 512], F32, tag="ctx", name=f"c2_{ci}")
                fill = list(fillers)

                acts = []
                bi = 0
                for t in range(T):
                    acts.append(("A", t))
                    if t % 2 == 1 and bi < len(Bg):
                        acts.append(("B", bi))
                        bi += 1
                while bi < len(Bg):
                    acts.append(("B", bi))
                    bi += 1

                def emit_scores(ev):
                    kind, i = ev
                    spt = ring_ps.tile([P, 1024], F32, tag="ring",
                                       name=f"sp{kind}{ci}_{i}")
                    if kind == "A":
                        t = i
                        nc.tensor.matmul(spt[:, 0:512],
                                         lhsT=KT0[0:DH, t * P:(t + 1) * P],
                                         rhs=QT0[0:DH, cq:cq + 512],
                                         start=True, stop=True)
                        nc.tensor.matmul(spt[:, 512:1024],
                                         lhsT=KT0[DH:P, t * P:(t + 1) * P],
                                         rhs=QT0[DH:P, cq:cq + 512],
                                         start=True, stop=True)
                    else:
                        tg = Bg[i]
                        nc.tensor.matmul(spt[:, 0:512],
                                         lhsT=KT1d[0:DH, tg[0] * P:(tg[0] + 1) * P],
                                         rhs=QT1d[0:DH, cq:cq + 512],
                                         start=True, stop=True)
                        if len(tg) == 2:
                            nc.tensor.matmul(spt[:, 512:1024],
                                             lhsT=KT1d[DH:P, tg[1] * P:(tg[1] + 1) * P],
                                             rhs=QT1d[DH:P, cq:cq + 512],
                                             start=True, stop=True)
                    return spt

                def emit_exp(ev, spt):
                    kind, i = ev
                    w = 1024
                    if kind == "B" and len(Bg[i]) == 1:
                        w = 512
                    es = espool.tile([P, 1024], BF, tag="es", name=f"es{kind}{ci}_{i}")
                    nc.scalar.activation(es[:, 0:w], spt[:, 0:w], AFT.Exp,
                                         bias=0.0, scale=1.0 / math.sqrt(DH))
                    return es

                def emit_pv(ev, es):
                    kind, i = ev
                    if kind == "A":
                        t = i
                        nc.tensor.matmul(ctx0[:, :], lhsT=VP[:, t, 0:P],
                                         rhs=es[:, 0:512],
                                         start=(t == 0), stop=(t == T - 1))
                        nc.tensor.matmul(ctx1[:, :], lhsT=VP[:, t, P:2 * P],
                                         rhs=es[:, 512:1024],
                                         start=(t == 0), stop=(t == T - 1))
                    else:
                        for j, t in enumerate(Bg[i]):
                            nc.tensor.matmul(ctx2[:, :], lhsT=VP[:, t, 2 * P:3 * P],
                                             rhs=es[:, j * 512:(j + 1) * 512],
                                             start=(t == 0), stop=(t == T - 1))

                sps = {0: emit_scores(acts[0])}
                for i, ev in enumerate(acts):
                    es = emit_exp(ev, sps.pop(i))
                    if i + 1 < len(acts):
                        sps[i + 1] = emit_scores(acts[i + 1])
                    emit_pv(ev, es)
                    if fill:
                        fill.pop(0)()
                while fill:
                    fill.pop(0)()
                normalize(ci, ctx0, ctx1, ctx2)

            fillers = {
                0: [(lambda t=t: v_proj(t)) for t in range(2, T)] + [lambda: q_proj(1, False)],
                1: [(lambda m=m: o_proj(m)) for m in range(0, 4)] + [lambda: q_proj(2, False)],
                2: [(lambda m=m: o_proj(m)) for m in range(4, 8)] + [lambda: q_proj(3, False)],
                3: [(lambda m=m: o_proj(m)) for m in range(8, 12)],
            }
            for ci in range(NQC):
                emit_chunk(ci, fillers[ci])
            for m in range(12, 16):
                o_proj(m)
            if DBG:
                nc.sync.dma_start(out=d_qt1d[:, :], in_=QT1d)
                nc.sync.dma_start(out=d_kt1d[:, :], in_=KT1d)
                nc.sync.dma_start(out=d_ctx2d[:, :], in_=CTX2d)
                nc.sync.dma_start(out=d_qt0[:, :], in_=QT0)
                nc.sync.dma_start(out=d_kt0[:, :], in_=KT0)
                nc.sync.dma_start(out=d_vp[:, :], in_=VP.rearrange("p t c -> p (t c)"))
                nc.sync.dma_start(out=d_ctx01[:, :], in_=CTX01)

    nc.compile()
    return nc


def _get_prog(KP):
    if KP not in _prog_cache:
        _prog_cache[KP] = _build_nc(KP)
    return _prog_cache[KP]


def _run(inputs, trace=False):
    import ml_dtypes
    from concourse.bass_utils import run_bass_kernel_spmd

    BF = ml_dtypes.bfloat16

    query = np.asarray(inputs["query"], dtype=np.float32)
    key = np.asarray(inputs["key"], dtype=np.float32)
    value = np.asarray(inputs["value"], dtype=np.float32)
    mask = np.asarray(inputs["mask"])
    Wq = np.asarray(inputs["Wq"], dtype=np.float32)
    bq = np.asarray(inputs["bq"], dtype=np.float32)
    Wk = np.asarray(inputs["Wk"], dtype=np.float32)
    bk = np.asarray(inputs["bk"], dtype=np.float32)
    Wv = np.asarray(inputs["Wv"], dtype=np.float32)
    bv = np.asarray(inputs["bv"], dtype=np.float32)
    Wo = np.asarray(inputs["Wo"], dtype=np.float32)
    bo = np.asarray(inputs["bo"], dtype=np.float32)

    idx = [np.nonzero(mask[b, 0, 0] != 0)[0] for b in range(B)]
    keff = [len(i) for i in idx]
    KP = max(P, ((max(keff) + P - 1) // P) * P)
    T = KP // P

    nc = _get_prog(KP)

    per_batch = {}
    for b in range(B):
        xqT = np.ascontiguousarray(query[b].T).astype(BF)
        xkT = np.zeros((DM, KP), dtype=BF)
        xkT[:, :keff[b]] = key[b][idx[b]].T.astype(BF)
        xvT = np.zeros((DM, KP), dtype=BF)
        xvT[:, :keff[b]] = value[b][idx[b]].T.astype(BF)
        vmf = np.zeros((KP,), dtype=np.float32)
        vmf[:keff[b]] = 1.0
        vm2 = np.ascontiguousarray(vmf.reshape(T, P).T)  # [128, T]
        per_batch[b] = (xqT, xkT, xvT, vm2)

    in_maps = []
    for core in range(NCORES):
        b, g = core // G, core % G
        xqT, xkT, xvT, vm2 = per_batch[b]
        sl = slice(g * DQ, (g + 1) * DQ)
        in_maps.append({
            "xqT": xqT,
            "xkT": xkT,
            "xvT": xvT,
            "wq": np.ascontiguousarray(
                Wq[:, sl].reshape(DM // P, P, DQ).transpose(1, 0, 2).reshape(P, -1)
            ).astype(BF),
            "wk": np.ascontiguousarray(
                Wk[:, sl].reshape(DM // P, P, DQ).transpose(1, 0, 2).reshape(P, -1)
            ).astype(BF),
            "wv": np.ascontiguousarray(
                Wv[:, sl].reshape(DM // P, P, DQ).transpose(1, 0, 2).reshape(P, -1)
            ).astype(BF),
            "wo": np.ascontiguousarray(Wo[sl, :]).astype(BF),
            "bq": np.ascontiguousarray(bq[sl].reshape(DQ, 1)),
            "bk": np.ascontiguousarray(bk[sl].reshape(DQ, 1)),
            "bv": np.ascontiguousarray(bv[sl].reshape(1, DQ)),
            "vm": vm2,
        })

    res = run_bass_kernel_spmd(nc, in_maps, list(range(NCORES)), trace=trace)

    outp = np.zeros((B, S, DM), dtype=np.float32)
    for core in range(NCORES):
        outp[core // G] += res.results[core]["out"]
    outp += bo.reshape(1, 1, DM)
    return outp, res


def kernel(**inputs) -> np.ndarray:
    out, _ = _run(inputs, trace=False)
    return out


if __name__ == "__main__":
    nc = _build_nc(1152)
    print("build OK")
